# revision 24
# baseline (speedup 1.0000x reference)
"""Trainium2 Bass kernel for nn_EuclideanDeconf (retrieval_knn).

Computes out = -mean((x[:, :, None] - W.T[None, :, :])**2, axis=1)
            = (2*x@W.T - ||x||^2 - ||w||^2) / D

Sharding: data-parallel over batch across 8 NeuronCores (512 rows each),
W replicated. Per core:
  - x loaded fp32 (for exact ||x||^2), cast to bf16 on DVE
  - W loaded via SWDGE cast-DMA straight to bf16
  - both operands PE-transposed to d-major bf16 tiles
  - cross term as bf16 tensor-core GEMM accumulated in fp32 PSUM
  - ||w||^2/2 folded into the GEMM as one extra K=1 contraction row
  - epilogue: one ScalarE activation per tile: out = psum*(2/D) - ||x||^2/D
"""

import os

import ml_dtypes
import numpy as np

B, D, C = 4096, 4096, 1024
NCORES = 8
P = 128
NW = 256  # output-tile free width (c); one PSUM bank holds 512 fp32
KG = 4    # transpose k-chunks per PSUM batch

_nc_cache = {}


def _interleave(n_x, n_w):
    """Merge x-tile and w-tile load order, x spread evenly among w."""
    items = []
    xi = wi = 0
    while xi < n_x or wi < n_w:
        if xi < n_x and (wi >= n_w or xi * n_w <= wi * n_x):
            items.append(("x", xi))
            xi += 1
        else:
            items.append(("w", wi))
            wi += 1
    return items


def _build_bass(b_sh, d, c):
    import concourse.bacc as bacc
    import concourse.mybir as mybir
    import concourse.tile as tile
    f32 = mybir.dt.float32
    bf16 = mybir.dt.bfloat16
    AF = mybir.ActivationFunctionType

    BT = b_sh // P    # b-tiles per core
    KC = d // P       # contraction chunks
    CT = c // P       # c-tiles
    NS = c // NW      # n-slices (output tile columns)
    CPN = NW // P     # c-tiles per n-slice
    NKG = KC // KG

    nc = bacc.Bacc(trn_type="TRN2")
    x_d = nc.dram_tensor("x", [b_sh, d], f32, kind="ExternalInput")
    w_d = nc.dram_tensor("W", [c, d], f32, kind="ExternalInput")
    id_d = nc.dram_tensor("ident", [P, P], bf16, kind="ExternalInput")
    o_d = nc.dram_tensor("out", [b_sh, c], f32, kind="ExternalOutput")

    with tile.TileContext(nc) as tc:
        with (
            tc.tile_pool(name="const", bufs=1) as constp,
            tc.tile_pool(name="persist", bufs=1) as persist,
            tc.tile_pool(name="xstage", bufs=4) as xstage,
            tc.tile_pool(name="wstage", bufs=2) as wstage,
            tc.tile_pool(name="bfs", bufs=3) as bfs,
            tc.tile_pool(name="outp", bufs=2) as outp,
            tc.tile_pool(name="trp", bufs=5, space="PSUM") as trp,
            tc.tile_pool(name="mmp", bufs=3, space="PSUM") as mmp,
            tc.tile_pool(name="dramp", bufs=2, space="DRAM") as dramp,
        ):
            ident = constp.tile([P, P], bf16)
            with tc.high_priority():
                nc.sync.dma_start(ident[:, :], id_d[:, :])
            wsrow = constp.tile([1, c], bf16)      # -||w_c||^2 / D, c-major
            ws_rep = persist.tile([P, c], bf16)    # wsrow broadcast to 128 p
            ws_cols = constp.tile([P, CT], f32)    # ||w||^2 per c-tile column
            ws_neg = constp.tile([P, CT], bf16)
            xs_bias = constp.tile([P, BT], f32)    # -||x_b||^2 / D per b-tile
            xTs = [
                persist.tile([P, KC, P], bf16, name=f"xT{i}") for i in range(BT)
            ]
            wTs = [
                persist.tile([P, KC, NW], bf16, name=f"wT{i}") for i in range(NS)
            ]
            HD = d // 2

            def transpose_into(src_bf, dst, col0, split_copies=False,
                               kg_range=None):
                # src_bf [P, d] bf16 (rows-major) -> dst[:, :, col0:col0+P]
                for kg in (kg_range if kg_range is not None else range(NKG)):
                    pt = trp.tile([P, KG, P], bf16, tag="tr")
                    for j in range(KG):
                        kk = kg * KG + j
                        nc.tensor.transpose(
                            pt[:, j, :], src_bf[:, kk * P : (kk + 1) * P], ident
                        )
                    dst_ap = dst[:, kg * KG : (kg + 1) * KG, col0 : col0 + P]
                    if split_copies and kg % 2 == 1:
                        nc.scalar.copy(dst_ap, pt[:, :, :])
                    else:
                        nc.vector.tensor_copy(out=dst_ap, in_=pt[:, :, :])

            def do_x(bt):
                with tc.high_priority():
                    xs = xstage.tile([P, d], f32, tag="xs")
                    nc.sync.dma_start(xs, x_d[bt * P : (bt + 1) * P, :])
                    xb = bfs.tile([P, d], bf16, tag="bfs")
                    nc.vector.tensor_copy(out=xb, in_=xs)
                    # square in place (fp32 dead after), free-dim sum
                    nc.scalar.activation(
                        xs, xs, AF.Square, accum_out=xs_bias[:, bt : bt + 1]
                    )
                    nc.vector.tensor_scalar_mul(
                        xs_bias[:, bt : bt + 1], xs_bias[:, bt : bt + 1], -1.0 / d
                    )
                transpose_into(xb, xTs[bt], 0)

            def do_w(ct):
                wb = bfs.tile([P, d], bf16, tag="bfs")
                for h in range(2):
                    with tc.high_priority():
                        ws_ = wstage.tile([P, HD], f32, tag="ws")
                        nc.gpsimd.dma_start(
                            ws_, w_d[ct * P : (ct + 1) * P, h * HD : (h + 1) * HD]
                        )
                        nc.vector.tensor_copy(
                            out=wb[:, h * HD : (h + 1) * HD], in_=ws_
                        )
                    transpose_into(
                        wb, wTs[ct // CPN], (ct % CPN) * P, split_copies=True,
                        kg_range=range(h * NKG // 2, (h + 1) * NKG // 2),
                    )
                with tc.high_priority():
                    # full-tile square from bf16, in place (wb dead after
                    # the transposes above), free-dim sum
                    nc.scalar.activation(
                        wb, wb, AF.Square, accum_out=ws_cols[:, ct : ct + 1]
                    )

            def build_wsrow(ns):
              with tc.high_priority():
                c0 = ns * CPN
                nc.vector.tensor_scalar_mul(
                    ws_neg[:, c0 : c0 + CPN], ws_cols[:, c0 : c0 + CPN], -1.0 / d
                )
                # cross-partition reshuffle [P, CPN] -> linear [1, NW] via DRAM
                dtmp = dramp.tile([CPN, P], bf16)
                for t in range(CPN):
                    nc.gpsimd.dma_start(dtmp[t, :], ws_neg[:, c0 + t : c0 + t + 1])
                nc.gpsimd.dma_start(wsrow[0:1, ns * NW : (ns + 1) * NW], dtmp[:, :])
                nc.gpsimd.partition_broadcast(
                    ws_rep[:, ns * NW : (ns + 1) * NW],
                    wsrow[0:1, ns * NW : (ns + 1) * NW],
                )

            def do_mm(bt, ns):
                ps = mmp.tile([P, NW], f32, tag="mm")
                for k in range(KC):
                    nc.tensor.matmul(
                        ps,
                        lhsT=xTs[bt][:, k, :],
                        rhs=wTs[ns][:, k, :],
                        start=(k == 0),
                        stop=(k == KC - 1),
                    )
                ot = outp.tile([P, NW], f32, tag="out")
                nc.vector.tensor_scalar(
                    ot, ps, 2.0 / d, xs_bias[:, bt : bt + 1],
                    mybir.AluOpType.mult, mybir.AluOpType.add,
                )
                nc.vector.tensor_tensor(
                    ot, ot, ws_rep[:, ns * NW : (ns + 1) * NW],
                    mybir.AluOpType.add,
                )
                nc.scalar.dma_start(
                    o_d[bt * P : (bt + 1) * P, ns * NW : (ns + 1) * NW], ot
                )

            x_loaded, w_loaded, ws_built, mm_done = set(), set(), set(), set()

            def emit_ready_mms():
                for ns in range(NS):
                    if ns not in ws_built:
                        continue
                    for bt in sorted(x_loaded):
                        if (bt, ns) not in mm_done:
                            do_mm(bt, ns)
                            mm_done.add((bt, ns))

            for kind, idx in _interleave(BT, CT):
                if kind == "x":
                    do_x(idx)
                    x_loaded.add(idx)
                else:
                    do_w(idx)
                    w_loaded.add(idx)
                    for ns in range(NS):
                        if ns not in ws_built and all(
                            t in w_loaded for t in range(ns * CPN, (ns + 1) * CPN)
                        ):
                            build_wsrow(ns)
                            ws_built.add(ns)
                emit_ready_mms()
            assert len(mm_done) == BT * NS

    nc.finalize()
    return nc


def _get_nc(b_sh, d, c):
    key = (b_sh, d, c)
    if key not in _nc_cache:
        _nc_cache[key] = _build_bass(b_sh, d, c)
    return _nc_cache[key]


last_result = None


def kernel(x, W):
    global last_result
    from concourse.bass_utils import run_bass_kernel_spmd

    x = np.ascontiguousarray(x, dtype=np.float32)
    W = np.ascontiguousarray(W, dtype=np.float32)
    b_sh = x.shape[0] // NCORES
    nc = _get_nc(b_sh, x.shape[1], W.shape[0])
    ident = np.eye(P, dtype=ml_dtypes.bfloat16)
    in_maps = [
        {
            "x": np.ascontiguousarray(x[i * b_sh : (i + 1) * b_sh]),
            "W": W,
            "ident": ident,
        }
        for i in range(NCORES)
    ]
    kw = {}
    if os.environ.get("KERNEL_TRACE", "0") == "1":
        cores = os.environ.get("KERNEL_TRACE_CORES", "0")
        kw = dict(trace=True, trace_cores=[int(t) for t in cores.split(",")])
    res = run_bass_kernel_spmd(nc, in_maps, core_ids=list(range(NCORES)), **kw)
    last_result = res
    return np.concatenate([res.results[i]["out"] for i in range(NCORES)], axis=0)


# revision 26
# speedup vs baseline: 1.2318x; 1.2318x over previous
"""Trainium2 Bass kernel for nn_EuclideanDeconf (retrieval_knn).

Computes out = -mean((x[:, :, None] - W.T[None, :, :])**2, axis=1)
            = (2*x@W.T - ||x||^2 - ||w||^2) / D

Sharding: data-parallel over batch across 8 NeuronCores (512 rows each),
W replicated. Per core the kernel computes out^T [C, B_sh] (the host
transposes back):
  - x loaded fp32 (for exact ||x||^2), cast to bf16, PE-transposed into a
    resident d-major xT [128, 32, 512] used as the matmul moving operand
  - W streamed: cast to bf16, each 128x128 block PE-transposed and used
    immediately as the stationary operand (no resident W^T)
  - cross term accumulated in fp32 PSUM as out^T tiles [128 c, 512 b]
  - epilogue per c-tile: (2/D)*psum - ||w_c||^2/D (per-partition scalar)
    - ||x||^2/D (broadcast row, built once via a tiny DRAM roundtrip +
    gpsimd partition_broadcast)
"""

import os

import ml_dtypes
import numpy as np

B, D, C = 4096, 4096, 1024
NCORES = 8
P = 128
KG = 4         # transpose k-chunks per PSUM batch (x side)
LOOKAHEAD = 3  # W-chunk pipeline depth before its matmuls

_nc_cache = {}


def _build_bass(b_sh, d, c):
    import concourse.bacc as bacc
    import concourse.mybir as mybir
    import concourse.tile as tile

    f32 = mybir.dt.float32
    bf16 = mybir.dt.bfloat16
    AF = mybir.ActivationFunctionType

    BT = b_sh // P    # b-tiles per core
    KC = d // P       # contraction chunks
    CT = c // P       # c-tiles
    NS = CT // 2      # c-tile pairs (one pass each)
    NKG = KC // KG
    HD = d // 2

    nc = bacc.Bacc(trn_type="TRN2")
    x_d = nc.dram_tensor("x", [b_sh, d], f32, kind="ExternalInput")
    w_d = nc.dram_tensor("W", [c, d], f32, kind="ExternalInput")
    id_d = nc.dram_tensor("ident", [P, P], bf16, kind="ExternalInput")
    o_d = nc.dram_tensor("out", [c, b_sh], f32, kind="ExternalOutput")

    with tile.TileContext(nc) as tc:
        with (
            tc.tile_pool(name="const", bufs=1) as constp,
            tc.tile_pool(name="persist", bufs=1) as persist,
            tc.tile_pool(name="xstage", bufs=2) as xstage,
            tc.tile_pool(name="wstage", bufs=6) as wstage,
            tc.tile_pool(name="bfs", bufs=5) as bfs,
            tc.tile_pool(name="wtc", bufs=2 * (LOOKAHEAD + 2)) as wtc,
            tc.tile_pool(name="outp", bufs=3) as outp,
            tc.tile_pool(name="trp", bufs=3, space="PSUM") as trp,
            tc.tile_pool(name="mmp", bufs=4, space="PSUM") as mmp,
            tc.tile_pool(name="dramp", bufs=2, space="DRAM") as dramp,
        ):
            ident = constp.tile([P, P], bf16)
            with tc.high_priority():
                nc.sync.dma_start(ident[:, :], id_d[:, :])
            ws_negD = constp.tile([P, CT], f32)   # -||w_c||^2/D, c-partition
            xs_cols = constp.tile([P, BT], f32)   # ||x_b||^2, b-partition
            xsrow = constp.tile([1, b_sh], f32)   # -||x||^2/D, b-major row
            xs_rep = persist.tile([P, b_sh], f32)  # xsrow on all partitions
            xT = persist.tile([P, KC, b_sh], bf16)

            # ---- x: load, square, cast, transpose into resident xT ----
            for bt in range(BT):
                with tc.high_priority():
                    xs = xstage.tile([P, d], f32, tag="xs")
                    nc.sync.dma_start(xs, x_d[bt * P : (bt + 1) * P, :])
                    xb = bfs.tile([P, d], bf16, tag="bfs")
                    nc.vector.tensor_copy(out=xb, in_=xs)
                    # square in place (fp32 dead after), free-dim sum
                    nc.scalar.activation(
                        xs, xs, AF.Square, accum_out=xs_cols[:, bt : bt + 1]
                    )
                for kg in range(NKG):
                    pt = trp.tile([P, KG, P], bf16, tag="tr")
                    for j in range(KG):
                        kk = kg * KG + j
                        nc.tensor.transpose(
                            pt[:, j, :], xb[:, kk * P : (kk + 1) * P], ident
                        )
                    dst = xT[:, kg * KG : (kg + 1) * KG, bt * P : (bt + 1) * P]
                    if kg % 2 == 1:
                        nc.scalar.copy(dst, pt[:, :, :])
                    else:
                        nc.vector.tensor_copy(out=dst, in_=pt[:, :, :])

            # ---- xs row: cols -> -xs/D -> DRAM -> row -> broadcast ----
            with tc.high_priority():
                nc.vector.tensor_scalar_mul(xs_cols, xs_cols, -1.0 / d)
                dtmp = dramp.tile([BT, P], f32)
                for t in range(BT):
                    nc.gpsimd.dma_start(dtmp[t, :], xs_cols[:, t : t + 1])
                nc.gpsimd.dma_start(xsrow[0:1, :], dtmp[:, :])
                nc.gpsimd.partition_broadcast(xs_rep[:, :], xsrow[0:1, :])

            # ---- W: stream pair-of-c-tiles passes ----
            def load_w_tile(ct):
                wb = bfs.tile([P, d], bf16, tag="bfs", name=f"wb{ct}")
                for h in range(2):
                    with tc.high_priority():
                        ws_ = wstage.tile([P, HD], f32, tag="ws")
                        nc.sync.dma_start(
                            ws_, w_d[ct * P : (ct + 1) * P, h * HD : (h + 1) * HD]
                        )
                        nc.vector.tensor_copy(
                            out=wb[:, h * HD : (h + 1) * HD], in_=ws_
                        )
                return wb

            for ns in range(NS):
                cts = (2 * ns, 2 * ns + 1)
                wbs = [load_w_tile(ct) for ct in cts]
                pss = [
                    mmp.tile([P, b_sh], f32, tag="mm", name=f"ps{ct}")
                    for ct in cts
                ]
                chunks = []

                def emit_chunk(k, wbs=wbs, chunks=chunks):
                    pt = trp.tile([P, 2, P], bf16, tag="tr")
                    for j in range(2):
                        nc.tensor.transpose(
                            pt[:, j, :], wbs[j][:, k * P : (k + 1) * P], ident
                        )
                    wt = wtc.tile([P, 2, P], bf16, tag="wtc")
                    if k % 2 == 1:
                        nc.scalar.copy(wt[:, :, :], pt[:, :, :])
                    else:
                        nc.vector.tensor_copy(out=wt[:, :, :], in_=pt[:, :, :])
                    chunks.append(wt)

                def emit_mms(k, pss=pss, chunks=chunks):
                    for j in range(2):
                        nc.tensor.matmul(
                            pss[j],
                            lhsT=chunks[k][:, j, :],
                            rhs=xT[:, k, :],
                            start=(k == 0),
                            stop=(k == KC - 1),
                        )

                for k in range(KC):
                    emit_chunk(k)
                    if k >= LOOKAHEAD:
                        emit_mms(k - LOOKAHEAD)
                for k in range(KC - LOOKAHEAD, KC):
                    emit_mms(k)

                for j, ct in enumerate(cts):
                    # ||w||^2 from bf16, in place (wb dead after transposes)
                    nc.scalar.activation(
                        wbs[j], wbs[j], AF.Square,
                        accum_out=ws_negD[:, ct : ct + 1],
                    )
                    nc.vector.tensor_scalar_mul(
                        ws_negD[:, ct : ct + 1], ws_negD[:, ct : ct + 1],
                        -1.0 / d,
                    )
                    ot = outp.tile([P, b_sh], f32, tag="out")
                    nc.vector.tensor_scalar(
                        ot, pss[j], 2.0 / d, ws_negD[:, ct : ct + 1],
                        mybir.AluOpType.mult, mybir.AluOpType.add,
                    )
                    nc.vector.tensor_tensor(
                        ot, ot, xs_rep, mybir.AluOpType.add
                    )
                    nc.scalar.dma_start(o_d[ct * P : (ct + 1) * P, :], ot)

    nc.finalize()
    return nc


def _get_nc(b_sh, d, c):
    key = (b_sh, d, c)
    if key not in _nc_cache:
        _nc_cache[key] = _build_bass(b_sh, d, c)
    return _nc_cache[key]


last_result = None


def kernel(x, W):
    global last_result
    from concourse.bass_utils import run_bass_kernel_spmd

    x = np.ascontiguousarray(x, dtype=np.float32)
    W = np.ascontiguousarray(W, dtype=np.float32)
    b_sh = x.shape[0] // NCORES
    nc = _get_nc(b_sh, x.shape[1], W.shape[0])
    ident = np.eye(P, dtype=ml_dtypes.bfloat16)
    in_maps = [
        {
            "x": np.ascontiguousarray(x[i * b_sh : (i + 1) * b_sh]),
            "W": W,
            "ident": ident,
        }
        for i in range(NCORES)
    ]
    kw = {}
    if os.environ.get("KERNEL_TRACE", "0") == "1":
        cores = os.environ.get("KERNEL_TRACE_CORES", "0")
        kw = dict(trace=True, trace_cores=[int(t) for t in cores.split(",")])
    res = run_bass_kernel_spmd(nc, in_maps, core_ids=list(range(NCORES)), **kw)
    last_result = res
    # per-core results are out^T [C, b_sh]; assemble and transpose back
    outT = np.concatenate([res.results[i]["out"] for i in range(NCORES)], axis=1)
    return np.ascontiguousarray(outT.T)


# revision 27
# speedup vs baseline: 1.3082x; 1.0621x over previous
"""Trainium2 Bass kernel for nn_EuclideanDeconf (retrieval_knn).

Computes out = -mean((x[:, :, None] - W.T[None, :, :])**2, axis=1)
            = (2*x@W.T - ||x||^2 - ||w||^2) / D

Sharding: data-parallel over batch across 8 NeuronCores (512 rows each),
W replicated. Per core the kernel computes out^T [C, B_sh] (the host
transposes back):
  - x loaded fp32 (for exact ||x||^2), cast to bf16, PE-transposed into a
    resident d-major xT [128, 32, 512] used as the matmul moving operand
  - W streamed: cast to bf16, each 128x128 block PE-transposed and used
    immediately as the stationary operand (no resident W^T)
  - cross term accumulated in fp32 PSUM as out^T tiles [128 c, 512 b]
  - epilogue per c-tile: (2/D)*psum - ||w_c||^2/D (per-partition scalar)
    - ||x||^2/D (broadcast row, built once via a tiny DRAM roundtrip +
    gpsimd partition_broadcast)
"""

import os

import ml_dtypes
import numpy as np

B, D, C = 4096, 4096, 1024
NCORES = 8
P = 128
KG = 4         # transpose k-chunks per PSUM batch (x side)
LOOKAHEAD = 6  # W-chunk pipeline depth before its matmuls

_nc_cache = {}


def _build_bass(b_sh, d, c):
    import concourse.bacc as bacc
    import concourse.mybir as mybir
    import concourse.tile as tile

    f32 = mybir.dt.float32
    bf16 = mybir.dt.bfloat16
    AF = mybir.ActivationFunctionType

    BT = b_sh // P    # b-tiles per core
    KC = d // P       # contraction chunks
    CT = c // P       # c-tiles
    NS = CT // 2      # c-tile pairs (one pass each)
    NKG = KC // KG
    HD = d // 2

    nc = bacc.Bacc(trn_type="TRN2")
    x_d = nc.dram_tensor("x", [b_sh, d], f32, kind="ExternalInput")
    w_d = nc.dram_tensor("W", [c, d], f32, kind="ExternalInput")
    id_d = nc.dram_tensor("ident", [P, P], bf16, kind="ExternalInput")
    o_d = nc.dram_tensor("out", [c, b_sh], f32, kind="ExternalOutput")

    with tile.TileContext(nc) as tc:
        with (
            tc.tile_pool(name="const", bufs=1) as constp,
            tc.tile_pool(name="persist", bufs=1) as persist,
            tc.tile_pool(name="xstage", bufs=2) as xstage,
            tc.tile_pool(name="wstage", bufs=6) as wstage,
            tc.tile_pool(name="bfs", bufs=5) as bfs,
            tc.tile_pool(name="wtc", bufs=2 * (LOOKAHEAD + 2)) as wtc,
            tc.tile_pool(name="outp", bufs=3) as outp,
            tc.tile_pool(name="trp", bufs=4, space="PSUM") as trp,
            tc.tile_pool(name="mmp", bufs=4, space="PSUM") as mmp,
            tc.tile_pool(name="dramp", bufs=2, space="DRAM") as dramp,
        ):
            ident = constp.tile([P, P], bf16)
            with tc.high_priority():
                nc.sync.dma_start(ident[:, :], id_d[:, :])
            ws_negD = constp.tile([P, CT], f32)   # -||w_c||^2/D, c-partition
            xs_cols = constp.tile([P, BT], f32)   # ||x_b||^2, b-partition
            xsrow = constp.tile([1, b_sh], f32)   # -||x||^2/D, b-major row
            xs_rep = persist.tile([P, b_sh], f32)  # xsrow on all partitions
            xT = persist.tile([P, KC, b_sh], bf16)

            # ---- x: load, square, cast, transpose into resident xT ----
            for bt in range(BT):
                with tc.high_priority():
                    xs = xstage.tile([P, d], f32, tag="xs")
                    nc.sync.dma_start(xs, x_d[bt * P : (bt + 1) * P, :])
                    xb = bfs.tile([P, d], bf16, tag="bfs")
                    nc.vector.tensor_copy(out=xb, in_=xs)
                    # square in place (fp32 dead after), free-dim sum
                    nc.scalar.activation(
                        xs, xs, AF.Square, accum_out=xs_cols[:, bt : bt + 1]
                    )
                for kg in range(NKG):
                    pt = trp.tile([P, KG, P], bf16, tag="tr")
                    for j in range(KG):
                        kk = kg * KG + j
                        nc.tensor.transpose(
                            pt[:, j, :], xb[:, kk * P : (kk + 1) * P], ident
                        )
                    dst = xT[:, kg * KG : (kg + 1) * KG, bt * P : (bt + 1) * P]
                    nc.vector.tensor_copy(out=dst, in_=pt[:, :, :])

            # ---- xs row: cols -> -xs/D -> DRAM -> row -> broadcast ----
            with tc.high_priority():
                nc.vector.tensor_scalar_mul(xs_cols, xs_cols, -1.0 / d)
                dtmp = dramp.tile([BT, P], f32)
                for t in range(BT):
                    nc.gpsimd.dma_start(dtmp[t, :], xs_cols[:, t : t + 1])
                nc.gpsimd.dma_start(xsrow[0:1, :], dtmp[:, :])
                nc.gpsimd.partition_broadcast(xs_rep[:, :], xsrow[0:1, :])

            # ---- W: stream pair-of-c-tiles passes ----
            def load_w_tile(ct):
                wb = bfs.tile([P, d], bf16, tag="bfs", name=f"wb{ct}")
                for h in range(2):
                    with tc.high_priority():
                        ws_ = wstage.tile([P, HD], f32, tag="ws")
                        nc.sync.dma_start(
                            ws_, w_d[ct * P : (ct + 1) * P, h * HD : (h + 1) * HD]
                        )
                        nc.vector.tensor_copy(
                            out=wb[:, h * HD : (h + 1) * HD], in_=ws_
                        )
                return wb

            for ns in range(NS):
                cts = (2 * ns, 2 * ns + 1)
                wbs = [load_w_tile(ct) for ct in cts]
                pss = [
                    mmp.tile([P, b_sh], f32, tag="mm", name=f"ps{ct}")
                    for ct in cts
                ]
                chunks = []

                def emit_chunk(k, wbs=wbs, chunks=chunks):
                    pt = trp.tile([P, 2, P], bf16, tag="tr")
                    for j in range(2):
                        nc.tensor.transpose(
                            pt[:, j, :], wbs[j][:, k * P : (k + 1) * P], ident
                        )
                    wt = wtc.tile([P, 2, P], bf16, tag="wtc")
                    nc.scalar.copy(wt[:, :, :], pt[:, :, :])
                    chunks.append(wt)

                def emit_mms(k, pss=pss, chunks=chunks):
                    for j in range(2):
                        nc.tensor.matmul(
                            pss[j],
                            lhsT=chunks[k][:, j, :],
                            rhs=xT[:, k, :],
                            start=(k == 0),
                            stop=(k == KC - 1),
                        )

                for k in range(KC):
                    emit_chunk(k)
                    if k >= LOOKAHEAD:
                        emit_mms(k - LOOKAHEAD)
                for k in range(KC - LOOKAHEAD, KC):
                    emit_mms(k)

                for j, ct in enumerate(cts):
                    # ||w||^2 from bf16, in place (wb dead after transposes)
                    nc.scalar.activation(
                        wbs[j], wbs[j], AF.Square,
                        accum_out=ws_negD[:, ct : ct + 1],
                    )
                    nc.vector.tensor_scalar_mul(
                        ws_negD[:, ct : ct + 1], ws_negD[:, ct : ct + 1],
                        -1.0 / d,
                    )
                    ot = outp.tile([P, b_sh], f32, tag="out")
                    nc.vector.tensor_scalar(
                        ot, pss[j], 2.0 / d, ws_negD[:, ct : ct + 1],
                        mybir.AluOpType.mult, mybir.AluOpType.add,
                    )
                    nc.vector.tensor_tensor(
                        ot, ot, xs_rep, mybir.AluOpType.add
                    )
                    nc.scalar.dma_start(o_d[ct * P : (ct + 1) * P, :], ot)

    nc.finalize()
    return nc


def _get_nc(b_sh, d, c):
    key = (b_sh, d, c)
    if key not in _nc_cache:
        _nc_cache[key] = _build_bass(b_sh, d, c)
    return _nc_cache[key]


last_result = None


def kernel(x, W):
    global last_result
    from concourse.bass_utils import run_bass_kernel_spmd

    x = np.ascontiguousarray(x, dtype=np.float32)
    W = np.ascontiguousarray(W, dtype=np.float32)
    b_sh = x.shape[0] // NCORES
    nc = _get_nc(b_sh, x.shape[1], W.shape[0])
    ident = np.eye(P, dtype=ml_dtypes.bfloat16)
    in_maps = [
        {
            "x": np.ascontiguousarray(x[i * b_sh : (i + 1) * b_sh]),
            "W": W,
            "ident": ident,
        }
        for i in range(NCORES)
    ]
    kw = {}
    if os.environ.get("KERNEL_TRACE", "0") == "1":
        cores = os.environ.get("KERNEL_TRACE_CORES", "0")
        kw = dict(trace=True, trace_cores=[int(t) for t in cores.split(",")])
    res = run_bass_kernel_spmd(nc, in_maps, core_ids=list(range(NCORES)), **kw)
    last_result = res
    # per-core results are out^T [C, b_sh]; assemble and transpose back
    outT = np.concatenate([res.results[i]["out"] for i in range(NCORES)], axis=1)
    return np.ascontiguousarray(outT.T)
